# revision 10
# baseline (speedup 1.0000x reference)
"""Chamfer-distance (nn_CDLossEval) Trainium2 kernel.

Problem: pcs1, pcs2: [16, 4096, 3] f32.
  d[b,n,m] = ||pcs1[b,n]-pcs2[b,m]||^2 (clamped >= 0)
  dist1 = min_m d, dist2 = min_n d, mean = mean(dist1)+mean(dist2)
Returns (mean, dist1, dist2) like the reference.

Strategy (8 cores, data-parallel over B: 2 batches/core):
  * Distance matrix via a single K=32 augmented matmul:
    d = aa + bb - 2 a.b expressed as sum_k L[k,n] * R[k,m] where L/R are
    bf16 3-way splits of the norms and coordinates (fp32-grade accuracy,
    full-rate bf16 matmul: 1 cycle/row).
  * Both reduction directions are computed as row-mins of the two matmul
    orientations (d and d^T), so all reductions are free-axis.
  * Per 128-row chunk the 4096 columns are produced into PSUM in
    [128,1024] halves (psA, psB). ScalarE copies psA -> SBUF; VectorE
    runs one fused tensor_tensor_reduce(min, min) pairing the SBUF copy
    with the PSUM half: 2048 fresh elements per 1024 DVE cycles, with the
    row-min accumulated via the chained scalar initializer.
  * Clamp to >= 0 after the min (max is monotone so this commutes).
  * Mean computed on host from the returned dist1/dist2 (trivially small).
"""

import numpy as np
import ml_dtypes

import concourse.bass as bass
import concourse.bacc as bacc
import concourse.tile as tile
from concourse import mybir
from concourse import bass_utils

bf16 = ml_dtypes.bfloat16


# ----------------------------------------------------------------------------
# Custom DVE op: out = min(in0, in1); accum_out = min(s0, min_k out)
# Registered once at import. The per-NEFF DVE table generator compiles the
# Spec to uOps, so this needs no firmware change (see dve_table_gen.py).
# One instruction consumes 2 fresh tensors at 1 result/cycle — 0.5 cyc/elem,
# the best min-reduction rate the DVE offers for fp32.
# ----------------------------------------------------------------------------

def _register_min2():
    import numpy as _np
    from concourse import dve_ops as _dv
    from concourse.dve_spec import Spec, Src0, Src1, C0, minn, lower
    from concourse.dve_ops import DveOp, DveOpSpec

    name = "MIN2_REDUCE_CD"
    if any(o.name == name for o in _dv.OPS):
        return next(o for o in _dv.OPS if o.name == name)

    def _ref(in0, in1, c0, c1, c2):
        out = _np.minimum(in0.astype(_np.float32), in1)
        acc = _np.minimum(
            _np.asarray(c0, dtype=_np.float32).reshape(-1, 1) if
            _np.ndim(c0) else _np.float32(c0),
            out.reshape(out.shape[0], -1).min(axis=1, keepdims=True))
        return out, _np.broadcast_to(acc, (out.shape[0], 1))

    spec = Spec(body=minn(Src0, Src1), accum=minn, accum_init=C0,
                reference=_ref)
    _dv._SUB_OPCODE_FOR_NAME[name] = 1 + len(_dv.OPS)
    shas = {}
    for ver in ("v3", "v4"):
        s = DveOpSpec(name=name, opcode=_dv._SUB_OPCODE_FOR_NAME[name],
                      uops=lower(spec, ver=ver), rd1_en=True)
        shas[ver] = s.sha(ver)
    op = DveOp(name, spec, subdim=False, uops_sha=shas)
    _dv.OPS.append(op)
    _dv.CUSTOM_DVE_SPECS[name] = spec
    return op


MIN2 = _register_min2()

B, NPTS, NDIM = 16, 4096, 3
N_CORES = 8
NB = B // N_CORES  # local batches per core
K = 32             # augmented contraction rows (30 used + 2 zero pad)
MMW = 512          # matmul moving width (one PSUM bank of f32)


# ----------------------------------------------------------------------------
# Host-side input augmentation
# ----------------------------------------------------------------------------

def _split3_f32(x):
    """f32 array -> 3 bf16 components with sum error ~2^-27 relative."""
    h = x.astype(bf16)
    r = x - h.astype(np.float32)
    m = r.astype(bf16)
    r2 = r - m.astype(np.float32)
    return h, m, r2.astype(bf16)


def _split3_f64(x):
    h = x.astype(bf16)
    r = x - h.astype(np.float64)
    m = r.astype(bf16)
    r2 = r - m.astype(np.float64)
    return h, m, r2.astype(bf16)


def build_forms(x):
    """x: [N,3] f32 -> (L, R) bf16 [K, N] lhs-form / rhs-form.

    sum_k L[k,i] * R'[k,j] over pairing with the other tensor's opposite
    form gives aa[i] + bb[j] - 2*x_i . y_j to ~fp32 accuracy:
      rows 0-2:  L = norm splits,  R = ones    (contributes aa)
      rows 3-5:  L = ones,         R = norm splits (contributes bb)
      rows 6-29: coordinate products of (-2x) splits against raw splits,
                 all 3x3 cross terms except lo*lo.
    """
    n = x.shape[0]
    nn = (x.astype(np.float64) ** 2).sum(-1)
    nh, nm, nl = _split3_f64(nn)
    c = -2.0 * x.astype(np.float32)           # exact scaling
    ch, cm, cl = _split3_f32(c)               # each [N,3]
    xh, xm, xl = _split3_f32(x.astype(np.float32))
    ones = np.ones(n, dtype=bf16)
    zeros = np.zeros(n, dtype=bf16)

    Lrows = [nh, nm, nl, ones, ones, ones]
    Rrows = [ones, ones, ones, nh, nm, nl]
    for k in range(NDIM):
        Lrows += [ch[:, k]] * 3 + [cm[:, k]] * 3 + [cl[:, k]] * 2
        Rrows += [xh[:, k], xm[:, k], xl[:, k]] * 2 + [xh[:, k], xm[:, k]]
    Lrows += [zeros, zeros]
    Rrows += [zeros, zeros]
    return np.stack(Lrows), np.stack(Rrows)


def prep_inputs(pcs1, pcs2):
    """Per batch: L/R forms of both point clouds. Returns dict of
    [B, K, N] bf16 arrays."""
    La = np.empty((pcs1.shape[0], K, pcs1.shape[1]), dtype=bf16)
    Ra = np.empty_like(La)
    Lb = np.empty_like(La)
    Rb = np.empty_like(La)
    for b in range(pcs1.shape[0]):
        La[b], Ra[b] = build_forms(np.asarray(pcs1[b], dtype=np.float32))
        Lb[b], Rb[b] = build_forms(np.asarray(pcs2[b], dtype=np.float32))
    return {"La": La, "Ra": Ra, "Lb": Lb, "Rb": Rb}


# ----------------------------------------------------------------------------
# Device program
# ----------------------------------------------------------------------------

def build_program(nc, nb=NB, npts=NPTS, reps=1):
    """Trace the SPMD per-core program into `nc`. DRAM tensors:
      in:  La, Rb, Lb, Ra   [nb*K, npts] bf16
      out: dist1, dist2     [nb*128, npts//128] f32  (partition-major layout)
    reps>1 wraps the compute in a For_i that re-executes it (timing only).
    """
    f32 = mybir.dt.float32
    nchunks = npts // 128
    AW = min(1024, npts // 2)        # psA/psB width
    steps = npts // (2 * AW)         # reduce steps per chunk
    mmA = AW // MMW                  # matmuls per half-step half

    dram_in = {
        name: nc.dram_tensor(name, [nb * K, npts], mybir.dt.bfloat16,
                             kind="ExternalInput").ap()
        for name in ("La", "Rb", "Lb", "Ra")
    }
    d1 = nc.dram_tensor("dist1", [nb * 128, nchunks], f32,
                        kind="ExternalOutput").ap()
    d2 = nc.dram_tensor("dist2", [nb * 128, nchunks], f32,
                        kind="ExternalOutput").ap()

    with tile.TileContext(nc) as tc:
        with (
            tc.tile_pool(name="weights", bufs=4 * nb) as wpool,
            tc.tile_pool(name="psum", bufs=2, space="PSUM") as ppool,
            tc.tile_pool(name="cp", bufs=3) as cpool,
            tc.tile_pool(name="rowmin", bufs=2) as rpool,
            tc.tile_pool(name="rmtmp", bufs=2) as tpool,
            tc.tile_pool(name="dummy", bufs=2) as dpool,
        ):
            # Load all augmented inputs up front.
            sb = {}
            for name, ap in dram_in.items():
                for lb in range(nb):
                    wt = wpool.tile([K, npts], mybir.dt.bfloat16)
                    nc.sync.dma_start(out=wt, in_=ap[lb * K:(lb + 1) * K, :])
                    sb[(name, lb)] = wt

            jobs = []
            for lb in range(nb):
                jobs.append((sb[("La", lb)], sb[("Rb", lb)], d1, lb))
                jobs.append((sb[("Lb", lb)], sb[("Ra", lb)], d2, lb))

            import contextlib
            loop_ctx = tc.For_i(0, reps, 1) if reps > 1 else contextlib.nullcontext()
            with loop_ctx:
                _emit_jobs(nc, tc, jobs, ppool, cpool, rpool, tpool, dpool)
    return nc


def _emit_jobs(nc, tc, jobs, ppool, cpool, rpool, tpool, dpool):
    f32 = mybir.dt.float32
    for lhs, rhs, dout, lb in jobs:
        npts = lhs.shape[1]
        nchunks = npts // 128
        AW = min(1024, npts // 2)
        steps = npts // (2 * AW)
        mmA = AW // MMW
        # per-(chunk, step) row-min partials; folded with one 3D reduce below
        parts = rpool.tile([128, nchunks * steps], f32, tag="parts")
        for c in range(nchunks):
            lhsT = lhs[:, c * 128:(c + 1) * 128]
            for s in range(steps):
                col0 = s * 2 * AW
                psA = ppool.tile([128, AW], f32, tag="psA")
                psB = ppool.tile([128, AW], f32, tag="psB")
                for j in range(mmA):
                    nc.tensor.matmul(
                        out=psA[:, j * MMW:(j + 1) * MMW],
                        lhsT=lhsT,
                        rhs=rhs[:, col0 + j * MMW:col0 + (j + 1) * MMW],
                        start=True, stop=True,
                    )
                for j in range(mmA):
                    nc.tensor.matmul(
                        out=psB[:, j * MMW:(j + 1) * MMW],
                        lhsT=lhsT,
                        rhs=rhs[:, col0 + AW + j * MMW:
                                col0 + AW + (j + 1) * MMW],
                        start=True, stop=True,
                    )
                cp = cpool.tile([128, AW], f32)
                nc.scalar.copy(out=cp, in_=psA)
                scratch = dpool.tile([128, AW], f32, tag="scratch")
                k = c * steps + s
                nc.vector._custom_dve(
                    MIN2, out=scratch, in0=cp, in1=psB,
                    s0=1e30, accum_out=parts[:, k:k + 1],
                )
        # fold step partials -> per-chunk row-min, clamp at 0, store
        rowmin = tpool.tile([128, nchunks], f32, tag="rowmin")
        nc.vector.tensor_reduce(
            out=rowmin,
            in_=parts.rearrange("p (c s) -> p c s", s=steps),
            axis=mybir.AxisListType.X,
            op=mybir.AluOpType.min)
        nc.vector.tensor_scalar_max(out=rowmin, in0=rowmin, scalar1=0.0)
        nc.sync.dma_start(out=dout[lb * 128:(lb + 1) * 128, :], in_=rowmin)
    return nc


_CACHED = {}


def _get_program(nb=NB, npts=NPTS):
    key = (nb, npts)
    if key not in _CACHED:
        nc = bacc.Bacc("TRN2", target_bir_lowering=False, debug=False)
        build_program(nc, nb=nb, npts=npts)
        nc.compile()
        _CACHED[key] = nc
    return _CACHED[key]


def _assemble(pc, nb, npts):
    """[nb*128, nchunks] partition-major -> [nb, npts]."""
    nchunks = npts // 128
    a = pc.reshape(nb, 128, nchunks)
    return np.ascontiguousarray(a.transpose(0, 2, 1)).reshape(nb, npts)


def kernel(pcs1, pcs2, _trace=False):
    pcs1 = np.asarray(pcs1, dtype=np.float32)
    pcs2 = np.asarray(pcs2, dtype=np.float32)
    forms = prep_inputs(pcs1, pcs2)

    nc = _get_program()
    in_maps = []
    for core in range(N_CORES):
        lo, hi = core * NB, (core + 1) * NB
        in_maps.append({
            name: np.ascontiguousarray(
                forms[name][lo:hi].reshape(NB * K, NPTS))
            for name in ("La", "Ra", "Lb", "Rb")
        })
    res = bass_utils.run_bass_kernel_spmd(
        nc, in_maps, core_ids=list(range(N_CORES)), trace=_trace)

    dist1 = np.empty((B, NPTS), dtype=np.float32)
    dist2 = np.empty((B, NPTS), dtype=np.float32)
    for core in range(N_CORES):
        lo, hi = core * NB, (core + 1) * NB
        dist1[lo:hi] = _assemble(res.results[core]["dist1"], NB, NPTS)
        dist2[lo:hi] = _assemble(res.results[core]["dist2"], NB, NPTS)

    mean_dist = np.float32(dist1.mean(dtype=np.float64)) + np.float32(
        dist2.mean(dtype=np.float64))
    if _trace:
        return (mean_dist, dist1, dist2), res
    return (mean_dist, dist1, dist2)
